# revision 1
# baseline (speedup 1.0000x reference)
"""HGRN attention Trainium2 kernel.

Sharding: B*L (4 batches x 4096 tokens) split into 8 chunks of T=2048 tokens,
one per NeuronCore: core c = 2*b + half handles tokens [half*T, (half+1)*T) of
batch b. The gated linear recurrence h_t = sigmoid(f_t)*h_{t-1} + swiglu-input
runs per (batch, channel); the cross-chunk carry (h at the half boundary) is
exchanged with a tiny pairwise AllReduce and applied as h_local + cumprod*carry
(cumprod underflows to exactly 0 in fp32 past ~130 steps, so only the first 256
columns of each odd chunk need the fixup - bit-matching the fp32 reference).

On-chip layout is transposed ([channel, time]) so the recurrence maps onto the
DVE tensor_tensor_scan instruction; the host pre-transposes x and the weights,
and the kernel emits y transposed (host transposes back). Matmuls run in
float32r (tf32-rate, ~1.5e-4 rel err). RMSNorm over channels uses a full
ONES[128x128] matmul for the cross-partition reduce+broadcast.
"""
import numpy as np

import concourse.bacc as bacc
import concourse.tile as tile
import concourse.mybir as mybir
from concourse.bass_utils import run_bass_kernel_spmd

B, L, D = 4, 4096, 2048
T = 2048                 # tokens per core
NCORE = 8
ET = DT = D // 128       # 16 tiles of 128 channels
TB1 = 1024               # phase-1 time block
NB1 = T // TB1
TB2 = 512                # phase-2/3 time block
NB2 = T // TB2
CLEN = 256               # cumprod fixup length (0 in fp32 beyond this)
EPS = 1e-5

F32 = mybir.dt.float32
F32R = mybir.dt.float32r
AF = mybir.ActivationFunctionType
OP = mybir.AluOpType

_CACHE = {}


def _build():
    nc = bacc.Bacc("TRN2", target_bir_lowering=False, debug=False,
                   enable_asserts=True, num_devices=NCORE)
    xt_d = nc.dram_tensor("xt", [D, T], F32R, kind="ExternalInput")
    wi_d = nc.dram_tensor("wi", [D, D], F32R, kind="ExternalInput")   # (d_in, e)
    wf_d = nc.dram_tensor("wf", [D, D], F32R, kind="ExternalInput")
    wg_d = nc.dram_tensor("wg", [D, D], F32R, kind="ExternalInput")
    wo_d = nc.dram_tensor("wo", [D, D], F32R, kind="ExternalInput")   # (e, d_out)
    gnw_d = nc.dram_tensor("gnw", [128, ET], F32, kind="ExternalInput")
    mask_d = nc.dram_tensor("mask", [128, 1], F32, kind="ExternalInput")
    yt_d = nc.dram_tensor("yt", [D, T], F32, kind="ExternalOutput")

    with tile.TileContext(nc) as tc:
        with tc.tile_pool(name="persist", bufs=1) as pp, \
             tc.tile_pool(name="dram", bufs=1, space="DRAM") as dr:
            carry = pp.tile([128, ET], F32, tag="carry")
            recv = pp.tile([128, ET], F32, tag="recv")
            cin = pp.tile([128, ET], F32, tag="cin")
            gnw = pp.tile([128, ET], F32, tag="gnw")
            maskt = pp.tile([128, 1], F32, tag="mask")
            acc = pp.tile([128, T], F32, tag="acc")
            call = pp.tile([128, ET * CLEN], F32, tag="call")
            rms = pp.tile([128, T], F32, tag="rms")
            ones = pp.tile([128, 128], F32, tag="ones")

            h_sp = dr.tile([D, T], F32, tag="hsp")
            g_sp = dr.tile([D, T], F32, tag="gsp")
            hl_i = dr.tile([128, ET], F32, tag="hli")
            hl_o = dr.tile([128, ET], F32, tag="hlo")

            nc.vector.memset(carry[:], 0.0)
            nc.vector.memset(ones[:], 1.0)
            nc.sync.dma_start(gnw[:], gnw_d.ap()[:])
            nc.sync.dma_start(maskt[:], mask_d.ap()[:])

            # ---------------- phase 1: projections + scan + spills ----------
            with tc.tile_pool(name="xtp", bufs=1) as xtp, \
                 tc.tile_pool(name="wp", bufs=2) as wp, \
                 tc.tile_pool(name="wk", bufs=2) as wk, \
                 tc.tile_pool(name="pj", bufs=1, space="PSUM") as pj:
                for tb in range(NB1):
                    ts0 = tb * TB1
                    xt = xtp.tile([128, DT * TB1], F32R, tag="xt")
                    for dt in range(DT):
                        nc.sync.dma_start(
                            xt[:, dt * TB1:(dt + 1) * TB1],
                            xt_d.ap()[dt * 128:(dt + 1) * 128, ts0:ts0 + TB1])
                    nc.vector.memset(acc[:, ts0:ts0 + TB1], 0.0)
                    for et in range(ET):
                        es = slice(et * 128, (et + 1) * 128)
                        wts = []
                        for nm, wd in (("wi", wi_d), ("wf", wf_d), ("wg", wg_d)):
                            w = wp.tile([128, DT * 128], F32R, tag=nm)
                            nc.sync.dma_start(
                                w[:].rearrange("p (dt e) -> p dt e", e=128),
                                wd.ap().rearrange("(dt p) e -> p dt e",
                                                  p=128)[:, :, es])
                            wts.append(w)
                        ps = {}
                        for nm, w in zip(("i", "f", "g"), wts):
                            p = pj.tile([128, TB1], F32, tag="p" + nm)
                            for n in range(TB1 // 512):
                                for dt in range(DT):
                                    nc.tensor.matmul(
                                        p[:, n * 512:(n + 1) * 512],
                                        w[:, dt * 128:(dt + 1) * 128],
                                        xt[:, dt * TB1 + n * 512:
                                           dt * TB1 + (n + 1) * 512],
                                        start=(dt == 0), stop=(dt == DT - 1))
                            ps[nm] = p
                        gate = wk.tile([128, TB1], F32, tag="gate")
                        nc.scalar.activation(gate[:], ps["f"][:], AF.Sigmoid)
                        sil = wk.tile([128, TB1], F32, tag="sil")
                        nc.scalar.activation(sil[:], ps["i"][:], AF.Silu)
                        omg = wk.tile([128, TB1], F32, tag="omg")
                        nc.vector.tensor_scalar(omg[:], gate[:], -1.0, 1.0,
                                                OP.mult, OP.add)
                        iin = wk.tile([128, TB1], F32, tag="iin")
                        nc.vector.tensor_mul(iin[:], omg[:], sil[:])
                        h1 = wk.tile([128, TB1], F32, tag="h1")
                        nc.vector.tensor_tensor_scan(
                            h1[:], gate[:], iin[:], carry[:, et:et + 1],
                            OP.mult, OP.add)
                        nc.vector.tensor_copy(carry[:, et:et + 1],
                                              h1[:, TB1 - 1:TB1])
                        if tb == 0:
                            nc.vector.tensor_tensor_scan(
                                call[:, et * CLEN:(et + 1) * CLEN],
                                gate[:, 0:CLEN], gate[:, 0:CLEN], 1.0,
                                OP.mult, OP.bypass)
                        g1 = wk.tile([128, TB1], F32, tag="g1")
                        nc.scalar.copy(g1[:], ps["g"][:])
                        sq = wk.tile([128, TB1], F32, tag="sq")
                        nc.scalar.activation(sq[:], ps["g"][:], AF.Square)
                        nc.vector.tensor_add(acc[:, ts0:ts0 + TB1],
                                             acc[:, ts0:ts0 + TB1], sq[:])
                        nc.sync.dma_start(
                            h_sp[et * 128:(et + 1) * 128, ts0:ts0 + TB1], h1[:])
                        nc.sync.dma_start(
                            g_sp[et * 128:(et + 1) * 128, ts0:ts0 + TB1], g1[:])

            # ---------------- phase 1.5: carry exchange + rmsnorm -----------
            nc.sync.dma_start(hl_i[:], carry[:])
            nc.gpsimd.collective_compute(
                "AllReduce", OP.add,
                replica_groups=[[0, 1], [2, 3], [4, 5], [6, 7]],
                ins=[hl_i.opt()], outs=[hl_o.opt()])
            nc.sync.dma_start(recv[:], hl_o[:])
            nc.vector.tensor_sub(recv[:], recv[:], carry[:])
            nc.vector.tensor_scalar(cin[:], recv[:], maskt[:, 0:1], None,
                                    OP.mult)

            with tc.tile_pool(name="sp", bufs=1, space="PSUM") as sp, \
                 tc.tile_pool(name="rwk", bufs=1) as rwk:
                S = sp.tile([128, T], F32, tag="S")
                for n in range(T // 512):
                    nc.tensor.matmul(S[:, n * 512:(n + 1) * 512], ones[:],
                                     acc[:, n * 512:(n + 1) * 512],
                                     start=True, stop=True)
                m = rwk.tile([128, T], F32, tag="m")
                nc.vector.tensor_scalar(m[:], S[:], 1.0 / D, EPS,
                                        OP.mult, OP.add)
                rec = rwk.tile([128, T], F32, tag="rec")
                nc.vector.reciprocal(rec[:], m[:])
                nc.scalar.activation(rms[:], rec[:], AF.Sqrt)

            # ---------------- phase 2+3: gating + output projection ---------
            with tc.tile_pool(name="op2", bufs=2) as op2, \
                 tc.tile_pool(name="outp", bufs=2) as outp, \
                 tc.tile_pool(name="wop", bufs=2) as wop, \
                 tc.tile_pool(name="yp", bufs=2, space="PSUM") as yp, \
                 tc.tile_pool(name="yo", bufs=2) as yo:
                for tb2 in range(NB2):
                    ts = tb2 * TB2
                    osb = outp.tile([128, ET * TB2], F32R, tag="osb")
                    for et in range(ET):
                        h2 = op2.tile([128, TB2], F32, tag="h2")
                        nc.sync.dma_start(
                            h2[:], h_sp[et * 128:(et + 1) * 128, ts:ts + TB2])
                        g2 = op2.tile([128, TB2], F32, tag="g2")
                        nc.sync.dma_start(
                            g2[:], g_sp[et * 128:(et + 1) * 128, ts:ts + TB2])
                        if tb2 == 0:
                            nc.vector.scalar_tensor_tensor(
                                h2[:, 0:CLEN],
                                call[:, et * CLEN:(et + 1) * CLEN],
                                cin[:, et:et + 1], h2[:, 0:CLEN],
                                OP.mult, OP.add)
                        sw = op2.tile([128, TB2], F32, tag="sw")
                        nc.scalar.activation(sw[:], h2[:], AF.Silu)
                        w1 = op2.tile([128, TB2], F32, tag="w1")
                        nc.vector.tensor_mul(w1[:], g2[:], rms[:, ts:ts + TB2])
                        nc.vector.scalar_tensor_tensor(
                            osb[:, et * TB2:(et + 1) * TB2], w1[:],
                            gnw[:, et:et + 1], sw[:], OP.mult, OP.mult)
                    for dt in range(DT):
                        wo = wop.tile([128, ET * 128], F32R, tag="wo")
                        nc.sync.dma_start(
                            wo[:].rearrange("p (et d) -> p et d", d=128),
                            wo_d.ap().rearrange("(et p) d -> p et d",
                                                p=128)[:, :, dt * 128:(dt + 1) * 128])
                        ypt = yp.tile([128, TB2], F32, tag="ypt")
                        for et in range(ET):
                            nc.tensor.matmul(
                                ypt[:], wo[:, et * 128:(et + 1) * 128],
                                osb[:, et * TB2:(et + 1) * TB2],
                                start=(et == 0), stop=(et == ET - 1))
                        ysb = yo.tile([128, TB2], F32, tag="ysb")
                        nc.scalar.copy(ysb[:], ypt[:])
                        nc.sync.dma_start(
                            yt_d.ap()[dt * 128:(dt + 1) * 128, ts:ts + TB2],
                            ysb[:])
    nc.compile()
    return nc


def _get_nc():
    if "nc" not in _CACHE:
        _CACHE["nc"] = _build()
    return _CACHE["nc"]


def kernel(hidden_states, Wi, Wf, Wg, g_norm_weight, Wo, **_unused):
    nc = _get_nc()
    wiT = np.ascontiguousarray(Wi.T)
    wfT = np.ascontiguousarray(Wf.T)
    wgT = np.ascontiguousarray(Wg.T)
    woT = np.ascontiguousarray(Wo.T)
    gnw = np.ascontiguousarray(
        np.asarray(g_norm_weight, np.float32).reshape(ET, 128).T)
    in_maps = []
    for c in range(NCORE):
        b, half = c // 2, c % 2
        xt = np.ascontiguousarray(
            hidden_states[b, half * T:(half + 1) * T, :].T)
        mask = np.full((128, 1), float(half), np.float32)
        in_maps.append({"xt": xt, "wi": wiT, "wf": wfT, "wg": wgT,
                        "wo": woT, "gnw": gnw, "mask": mask})
    res = run_bass_kernel_spmd(nc, in_maps, list(range(NCORE))).results
    y = np.empty((B, L, D), np.float32)
    for c in range(NCORE):
        b, half = c // 2, c % 2
        y[b, half * T:(half + 1) * T, :] = res[c]["yt"].T
    return y



# revision 3
# speedup vs baseline: 1.4743x; 1.4743x over previous
"""HGRN attention Trainium2 kernel (bf16, fused, no spill).

Sharding: B*L (4 batches x 4096 tokens) split into 8 chunks of T=2048 tokens,
one per NeuronCore: core c = 2*b + half handles tokens [half*T, (half+1)*T) of
batch b. The gated linear recurrence h_t = sigmoid(f_t)*h_{t-1} + swiglu-input
runs per (batch, channel); the cross-chunk carry (h at the half boundary) is
exchanged with a tiny pairwise AllReduce and applied as h_local + cumprod*carry
(the fp32 cumprod of gates underflows to ~0 within ~130 steps, so only the
first CLEN=128 columns of each odd chunk need the fixup).

Everything runs in bf16 on the PE array (same 1 cycle/row rate as fp32r but
half the DMA/SBUF). On-chip layout is transposed ([channel, time]) so the
recurrence maps onto the DVE tensor_tensor_scan instruction. Key structural
trick: since rmsnorm's rms[t] is a per-token scalar, y = Wo.T @ (u * rms)
= (Wo.T @ u) * rms with u = g * gnw * silu(h) computed inline in phase A and
kept SBUF-resident in bf16 — neither h nor g is ever materialized to DRAM.
Phase B is pure matmul + a per-column rms scale. The carry-fixup time block
(tb=0) is processed last so the AllReduce latency hides under the other
blocks' matmuls. Weights are pre-shuffled on the host so every weight DMA is
fully contiguous and each weight byte is loaded exactly once.
"""
import numpy as np
import ml_dtypes

import concourse.bacc as bacc
import concourse.tile as tile
import concourse.mybir as mybir
from concourse.bass_utils import run_bass_kernel_spmd

B, L, D = 4, 4096, 2048
T = 2048                 # tokens per core
NCORE = 8
ET = DT = D // 128       # 16 tiles of 128 channels
TB = 512                 # time block (phase A scan block == phase B out block)
NB = T // TB             # 4
CLEN = 128               # cumprod fixup length (gate cumprod ~0 beyond this)
EPS = 1e-5

F32 = mybir.dt.float32
BF16 = mybir.dt.bfloat16
AF = mybir.ActivationFunctionType
OP = mybir.AluOpType

_CACHE = {}


def _build():
    nc = bacc.Bacc("TRN2", target_bir_lowering=False, debug=False,
                   enable_asserts=True, num_devices=NCORE)
    xt_d = nc.dram_tensor("xt", [D, T], BF16, kind="ExternalInput")
    # w{i,f,g}: host-prepped so row block et*128+p, col dt*128+e holds
    # W.T[dt*128+p, et*128+e]  (p = contraction index within dt block)
    wi_d = nc.dram_tensor("wi", [D, D], BF16, kind="ExternalInput")
    wf_d = nc.dram_tensor("wf", [D, D], BF16, kind="ExternalInput")
    wg_d = nc.dram_tensor("wg", [D, D], BF16, kind="ExternalInput")
    # wo: row block dt*128+pe, col et*128+d holds Wo.T[et*128+pe, dt*128+d]
    wo_d = nc.dram_tensor("wo", [D, D], BF16, kind="ExternalInput")
    gnw_d = nc.dram_tensor("gnw", [128, ET], F32, kind="ExternalInput")
    mask_d = nc.dram_tensor("mask", [128, 1], F32, kind="ExternalInput")
    yt_d = nc.dram_tensor("yt", [D, T], F32, kind="ExternalOutput")

    with tile.TileContext(nc) as tc:
        with tc.tile_pool(name="persist", bufs=1) as pp, \
             tc.tile_pool(name="dram", bufs=1, space="DRAM") as dr:
            carry = pp.tile([128, ET], F32, tag="carry")
            recv = pp.tile([128, ET], F32, tag="recv")
            cin = pp.tile([128, ET], F32, tag="cin")
            gnw = pp.tile([128, ET], F32, tag="gnw")
            maskt = pp.tile([128, 1], F32, tag="mask")
            ones = pp.tile([128, 128], F32, tag="ones")
            acc = pp.tile([128, T], F32, tag="acc")
            call = pp.tile([128, ET * CLEN], BF16, tag="call")
            haux = pp.tile([128, ET * CLEN], BF16, tag="haux")
            gaux = pp.tile([128, ET * CLEN], BF16, tag="gaux")
            ublk = [pp.tile([128, ET * TB], BF16, tag=f"u{n}", name=f"u{n}")
                    for n in range(NB)]

            hl_i = dr.tile([128, ET], F32, tag="hli")
            hl_o = dr.tile([128, ET], F32, tag="hlo")

            nc.vector.memset(carry[:], 0.0)
            nc.vector.memset(ones[:], 1.0)
            nc.vector.memset(acc[:], 0.0)
            nc.sync.dma_start(gnw[:], gnw_d.ap()[:])
            nc.sync.dma_start(maskt[:], mask_d.ap()[:])

            # ------------- phase A: projections + scan + gating -------------
            with tc.tile_pool(name="xp", bufs=1) as xp, \
                 tc.tile_pool(name="wp", bufs=2) as wp, \
                 tc.tile_pool(name="wk", bufs=2) as wk, \
                 tc.tile_pool(name="pj", bufs=2, space="PSUM") as pj:

                def load_w(et):
                    es = slice(et * 128, (et + 1) * 128)
                    wts = {}
                    for nm, wd in (("i", wi_d), ("f", wf_d), ("g", wg_d)):
                        w = wp.tile([128, DT * 128], BF16, tag="w" + nm)
                        nc.sync.dma_start(w[:], wd.ap()[es, :])
                        wts[nm] = w
                    return wts

                wts_next = load_w(0)
                xt = xp.tile([128, DT * T], BF16, tag="xt")
                for dt in range(DT):
                    nc.sync.dma_start(xt[:, dt * T:(dt + 1) * T],
                                      xt_d.ap()[dt * 128:(dt + 1) * 128, :])

                for et in range(ET):
                    wts = wts_next
                    if et + 1 < ET:
                        wts_next = load_w(et + 1)
                    for tb in range(NB):
                        t0 = tb * TB
                        ps = {}
                        for nm in ("i", "f", "g"):
                            p = pj.tile([128, TB], F32, tag="p" + nm)
                            for dt in range(DT):
                                nc.tensor.matmul(
                                    p[:], wts[nm][:, dt * 128:(dt + 1) * 128],
                                    xt[:, dt * T + t0:dt * T + t0 + TB],
                                    start=(dt == 0), stop=(dt == DT - 1))
                            ps[nm] = p
                        gate = wk.tile([128, TB], F32, tag="gate")
                        nc.scalar.activation(gate[:], ps["f"][:], AF.Sigmoid)
                        omg = wk.tile([128, TB], F32, tag="omg")
                        nc.scalar.activation(omg[:], ps["f"][:], AF.Sigmoid,
                                             scale=-1.0)
                        sil = wk.tile([128, TB], F32, tag="sil")
                        nc.scalar.activation(sil[:], ps["i"][:], AF.Silu)
                        iin = wk.tile([128, TB], F32, tag="iin")
                        nc.vector.tensor_mul(iin[:], omg[:], sil[:])
                        h1 = wk.tile([128, TB], F32, tag="h1")
                        nc.vector.tensor_tensor_scan(
                            h1[:], gate[:], iin[:], carry[:, et:et + 1],
                            OP.mult, OP.add)
                        nc.vector.tensor_copy(carry[:, et:et + 1],
                                              h1[:, TB - 1:TB])
                        siluh = wk.tile([128, TB], F32, tag="siluh")
                        nc.scalar.activation(siluh[:], h1[:], AF.Silu)
                        nc.vector.scalar_tensor_tensor(
                            ublk[tb][:, et * TB:(et + 1) * TB],
                            ps["g"][:], gnw[:, et:et + 1], siluh[:],
                            OP.mult, OP.mult)
                        sq = wk.tile([128, TB], F32, tag="sq")
                        nc.scalar.activation(sq[:], ps["g"][:], AF.Square)
                        nc.vector.tensor_add(acc[:, t0:t0 + TB],
                                             acc[:, t0:t0 + TB], sq[:])
                        if tb == 0:
                            cs = slice(et * CLEN, (et + 1) * CLEN)
                            nc.vector.tensor_tensor_scan(
                                call[:, cs], gate[:, 0:CLEN], gate[:, 0:CLEN],
                                1.0, OP.mult, OP.bypass)
                            nc.vector.tensor_copy(haux[:, cs], h1[:, 0:CLEN])
                            nc.scalar.copy(gaux[:, cs], ps["g"][:, 0:CLEN])

            # carry exchange: issue collective ASAP; defer recv consumption
            nc.sync.dma_start(hl_i[:], carry[:])
            nc.gpsimd.collective_compute(
                "AllReduce", OP.add,
                replica_groups=[[0, 1], [2, 3], [4, 5], [6, 7]],
                ins=[hl_i.opt()], outs=[hl_o.opt()])

            # ------------- phase B: rmsnorm scale + output projection -------
            with tc.tile_pool(name="rp", bufs=1) as rp, \
                 tc.tile_pool(name="sp", bufs=1, space="PSUM") as sp, \
                 tc.tile_pool(name="wop", bufs=2) as wop, \
                 tc.tile_pool(name="yp", bufs=2, space="PSUM") as yp, \
                 tc.tile_pool(name="yo", bufs=2) as yo, \
                 tc.tile_pool(name="fx", bufs=2) as fx:
                S = sp.tile([128, T], F32, tag="S")
                for n in range(T // 512):
                    nc.tensor.matmul(S[:, n * 512:(n + 1) * 512], ones[:],
                                     acc[:, n * 512:(n + 1) * 512],
                                     start=True, stop=True)
                m = rp.tile([128, T], F32, tag="m")
                nc.vector.tensor_scalar(m[:], S[:], 1.0 / D, EPS,
                                        OP.mult, OP.add)
                rec = rp.tile([128, T], F32, tag="rec")
                nc.vector.reciprocal(rec[:], m[:])
                rms = rp.tile([128, T], F32, tag="rms")
                nc.scalar.activation(rms[:], rec[:], AF.Sqrt)

                def outproj(dt, tbs, wo_t):
                    for tb2 in tbs:
                        ypt = yp.tile([128, TB], F32, tag="ypt")
                        for et in range(ET):
                            nc.tensor.matmul(
                                ypt[:], wo_t[:, et * 128:(et + 1) * 128],
                                ublk[tb2][:, et * TB:(et + 1) * TB],
                                start=(et == 0), stop=(et == ET - 1))
                        ysb = yo.tile([128, TB], F32, tag="ysb")
                        nc.vector.tensor_mul(ysb[:], ypt[:],
                                             rms[:, tb2 * TB:(tb2 + 1) * TB])
                        nc.sync.dma_start(
                            yt_d.ap()[dt * 128:(dt + 1) * 128,
                                      tb2 * TB:(tb2 + 1) * TB], ysb[:])

                # blocks 1..3 first: they don't need the carry fixup
                for dt in range(DT):
                    wo_t = wop.tile([128, ET * 128], BF16, tag="wo")
                    nc.sync.dma_start(wo_t[:],
                                      wo_d.ap()[dt * 128:(dt + 1) * 128, :])
                    outproj(dt, (1, 2, 3), wo_t)

                # consume the AllReduce (emitted late so in-order queues
                # don't stall phase B behind the collective)
                nc.sync.dma_start(recv[:], hl_o[:])
                nc.vector.tensor_sub(recv[:], recv[:], carry[:])
                nc.vector.tensor_scalar(cin[:], recv[:], maskt[:, 0:1], None,
                                        OP.mult)
                for et in range(ET):
                    cs = slice(et * CLEN, (et + 1) * CLEN)
                    hf = fx.tile([128, CLEN], F32, tag="hf")
                    nc.vector.scalar_tensor_tensor(
                        hf[:], call[:, cs], cin[:, et:et + 1], haux[:, cs],
                        OP.mult, OP.add)
                    sf = fx.tile([128, CLEN], F32, tag="sf")
                    nc.scalar.activation(sf[:], hf[:], AF.Silu)
                    nc.vector.scalar_tensor_tensor(
                        ublk[0][:, et * TB:et * TB + CLEN],
                        gaux[:, cs], gnw[:, et:et + 1], sf[:],
                        OP.mult, OP.mult)

                # block 0 last: reads the fixed-up ublk[0]
                for dt in range(DT):
                    wo_t = wop.tile([128, ET * 128], BF16, tag="wo")
                    nc.sync.dma_start(wo_t[:],
                                      wo_d.ap()[dt * 128:(dt + 1) * 128, :])
                    outproj(dt, (0,), wo_t)
    nc.compile()
    return nc


def _get_nc():
    if "nc" not in _CACHE:
        _CACHE["nc"] = _build()
    return _CACHE["nc"]


def _prep(wT):
    """[D, D] fp32 (already W.T) -> contiguous bf16 tiles.

    out[a*128+p, b*128+c] = wT[b*128+p, a*128+c]: block-transposed so a DMA of
    row block `a` yields the [128, DT*128] stationary tile for output block a.
    """
    return np.ascontiguousarray(
        np.asarray(wT, np.float32).reshape(DT, 128, ET, 128)
        .transpose(2, 1, 0, 3).reshape(D, D).astype(ml_dtypes.bfloat16))


def kernel(hidden_states, Wi, Wf, Wg, g_norm_weight, Wo, **_unused):
    nc = _get_nc()
    wi = _prep(np.asarray(Wi, np.float32).T)
    wf = _prep(np.asarray(Wf, np.float32).T)
    wg = _prep(np.asarray(Wg, np.float32).T)
    wo = _prep(np.asarray(Wo, np.float32).T)
    gnw = np.ascontiguousarray(
        np.asarray(g_norm_weight, np.float32).reshape(ET, 128).T)
    in_maps = []
    for c in range(NCORE):
        b, half = c // 2, c % 2
        xt = np.ascontiguousarray(
            np.asarray(hidden_states, np.float32)[b, half * T:(half + 1) * T, :].T
        ).astype(ml_dtypes.bfloat16)
        mask = np.full((128, 1), float(half), np.float32)
        in_maps.append({"xt": xt, "wi": wi, "wf": wf, "wg": wg,
                        "wo": wo, "gnw": gnw, "mask": mask})
    _CACHE["in_maps"] = in_maps
    res = run_bass_kernel_spmd(nc, in_maps, list(range(NCORE))).results
    y = np.empty((B, L, D), np.float32)
    for c in range(NCORE):
        b, half = c // 2, c % 2
        y[b, half * T:(half + 1) * T, :] = res[c]["yt"].T
    return y


# revision 5
# speedup vs baseline: 1.5202x; 1.0311x over previous
"""HGRN attention Trainium2 kernel (bf16, fused, no spill).

Sharding: B*L (4 batches x 4096 tokens) split into 8 chunks of T=2048 tokens,
one per NeuronCore: core c = 2*b + half handles tokens [half*T, (half+1)*T) of
batch b. The gated linear recurrence h_t = sigmoid(f_t)*h_{t-1} + swiglu-input
runs per (batch, channel); the cross-chunk carry (h at the half boundary) is
exchanged with a tiny pairwise AllReduce and applied as h_local + cumprod*carry
(the fp32 cumprod of gates underflows to ~0 within ~130 steps, so only the
first CLEN=128 columns of each odd chunk need the fixup).

Everything runs in bf16 on the PE array (same 1 cycle/row rate as fp32r but
half the DMA/SBUF). On-chip layout is transposed ([channel, time]) so the
recurrence maps onto the DVE tensor_tensor_scan instruction. Key structural
trick: since rmsnorm's rms[t] is a per-token scalar, y = Wo.T @ (u * rms)
= (Wo.T @ u) * rms with u = g * gnw * silu(h) computed inline in phase A and
kept SBUF-resident in bf16 — neither h nor g is ever materialized to DRAM.
Phase B is pure matmul + a per-column rms scale. The carry-fixup time block
(tb=0) is processed last so the AllReduce latency hides under the other
blocks' matmuls. Weights are pre-shuffled on the host so every weight DMA is
fully contiguous and each weight byte is loaded exactly once. All silu(x) are
computed as x*sigmoid(x) so the scalar engine keeps one activation table
loaded (no ACT_TABLE_LOAD thrash); rms uses the fused Rsqrt activation.
"""
import numpy as np
import ml_dtypes

import concourse.bacc as bacc
import concourse.tile as tile
import concourse.mybir as mybir
from concourse.bass_utils import run_bass_kernel_spmd

B, L, D = 4, 4096, 2048
T = 2048                 # tokens per core
NCORE = 8
ET = DT = D // 128       # 16 tiles of 128 channels
TB = 512                 # time block (phase A scan block == phase B out block)
NB = T // TB             # 4
CLEN = 128               # cumprod fixup length (gate cumprod ~0 beyond this)
EPS = 1e-5

F32 = mybir.dt.float32
BF16 = mybir.dt.bfloat16
AF = mybir.ActivationFunctionType
OP = mybir.AluOpType

_CACHE = {}


def _build():
    nc = bacc.Bacc("TRN2", target_bir_lowering=False, debug=False,
                   enable_asserts=True, num_devices=NCORE)
    xt_d = nc.dram_tensor("xt", [D, T], BF16, kind="ExternalInput")
    # w{i,f,g}: host-prepped so row block et*128+p, col dt*128+e holds
    # W.T[dt*128+p, et*128+e]  (p = contraction index within dt block)
    wi_d = nc.dram_tensor("wi", [D, D], BF16, kind="ExternalInput")
    wf_d = nc.dram_tensor("wf", [D, D], BF16, kind="ExternalInput")
    wg_d = nc.dram_tensor("wg", [D, D], BF16, kind="ExternalInput")
    # wo: row block dt*128+pe, col et*128+d holds Wo.T[et*128+pe, dt*128+d]
    wo_d = nc.dram_tensor("wo", [D, D], BF16, kind="ExternalInput")
    gnw_d = nc.dram_tensor("gnw", [128, ET], F32, kind="ExternalInput")
    mask_d = nc.dram_tensor("mask", [128, 1], F32, kind="ExternalInput")
    yt_d = nc.dram_tensor("yt", [D, T], F32, kind="ExternalOutput")

    with tile.TileContext(nc) as tc:
        with tc.tile_pool(name="persist", bufs=1) as pp, \
             tc.tile_pool(name="dram", bufs=1, space="DRAM") as dr:
            carry = pp.tile([128, ET], F32, tag="carry")
            recv = pp.tile([128, ET], F32, tag="recv")
            cin = pp.tile([128, ET], F32, tag="cin")
            gnw = pp.tile([128, ET], F32, tag="gnw")
            maskt = pp.tile([128, 1], F32, tag="mask")
            ones = pp.tile([128, 128], F32, tag="ones")
            acc = pp.tile([128, T], F32, tag="acc")
            call = pp.tile([128, ET * CLEN], BF16, tag="call")
            haux = pp.tile([128, ET * CLEN], BF16, tag="haux")
            gaux = pp.tile([128, ET * CLEN], BF16, tag="gaux")
            ublk = [pp.tile([128, ET * TB], BF16, tag=f"u{n}", name=f"u{n}")
                    for n in range(NB)]

            hl_i = dr.tile([128, ET], F32, tag="hli")
            hl_o = dr.tile([128, ET], F32, tag="hlo")

            nc.vector.memset(ones[:], 1.0)
            nc.vector.memset(acc[:], 0.0)
            nc.sync.dma_start(gnw[:], gnw_d.ap()[:])
            nc.sync.dma_start(maskt[:], mask_d.ap()[:])

            # ------------- phase A: projections + scan + gating -------------
            with tc.tile_pool(name="xp", bufs=1) as xp, \
                 tc.tile_pool(name="wp", bufs=2) as wp, \
                 tc.tile_pool(name="wk", bufs=2) as wk, \
                 tc.tile_pool(name="pj", bufs=2, space="PSUM") as pj:

                def load_w(et):
                    es = slice(et * 128, (et + 1) * 128)
                    wts = {}
                    for nm, wd in (("i", wi_d), ("f", wf_d), ("g", wg_d)):
                        w = wp.tile([128, DT * 128], BF16, tag="w" + nm)
                        nc.sync.dma_start(w[:], wd.ap()[es, :])
                        wts[nm] = w
                    return wts

                wts_next = load_w(0)
                # x chunked tb-major so the first psum group can start after
                # only 2MB has landed instead of 8MB
                xt = xp.tile([128, DT * T], BF16, tag="xt")
                for tb in range(NB):
                    for dt in range(DT):
                        c0 = dt * T + tb * TB
                        nc.sync.dma_start(
                            xt[:, c0:c0 + TB],
                            xt_d.ap()[dt * 128:(dt + 1) * 128,
                                      tb * TB:(tb + 1) * TB])

                for et in range(ET):
                    wts = wts_next
                    if et + 1 < ET:
                        wts_next = load_w(et + 1)
                    h_prev = None
                    for tb in range(NB):
                        t0 = tb * TB
                        ps = {}
                        for nm in ("i", "f", "g"):
                            p = pj.tile([128, TB], F32, tag="p" + nm)
                            for dt in range(DT):
                                nc.tensor.matmul(
                                    p[:], wts[nm][:, dt * 128:(dt + 1) * 128],
                                    xt[:, dt * T + t0:dt * T + t0 + TB],
                                    start=(dt == 0), stop=(dt == DT - 1))
                            ps[nm] = p
                        gate = wk.tile([128, TB], F32, tag="gate")
                        nc.scalar.activation(gate[:], ps["f"][:], AF.Sigmoid)
                        omg = wk.tile([128, TB], F32, tag="omg")
                        nc.scalar.activation(omg[:], ps["f"][:], AF.Sigmoid,
                                             scale=-1.0)
                        sigi = wk.tile([128, TB], F32, tag="sigi")
                        nc.scalar.activation(sigi[:], ps["i"][:], AF.Sigmoid)
                        isil = wk.tile([128, TB], F32, tag="isil")
                        nc.vector.tensor_mul(isil[:], ps["i"][:], sigi[:])
                        iin = wk.tile([128, TB], F32, tag="iin")
                        nc.vector.tensor_mul(iin[:], omg[:], isil[:])
                        h1 = wk.tile([128, TB], F32, tag="h1")
                        nc.vector.tensor_tensor_scan(
                            h1[:], gate[:], iin[:],
                            0.0 if tb == 0 else h_prev[:, TB - 1:TB],
                            OP.mult, OP.add)
                        sigh = wk.tile([128, TB], F32, tag="omg")
                        nc.scalar.activation(sigh[:], h1[:], AF.Sigmoid)
                        hs = wk.tile([128, TB], F32, tag="sigi")
                        nc.vector.tensor_mul(hs[:], h1[:], sigh[:])
                        nc.vector.scalar_tensor_tensor(
                            ublk[tb][:, et * TB:(et + 1) * TB],
                            ps["g"][:], gnw[:, et:et + 1], hs[:],
                            OP.mult, OP.mult)
                        sq = wk.tile([128, TB], F32, tag="gate")
                        nc.scalar.activation(sq[:], ps["g"][:], AF.Square)
                        nc.vector.tensor_add(acc[:, t0:t0 + TB],
                                             acc[:, t0:t0 + TB], sq[:])
                        if tb == NB - 1:
                            nc.vector.tensor_copy(carry[:, et:et + 1],
                                                  h1[:, TB - 1:TB])
                        if tb == 0:
                            cs = slice(et * CLEN, (et + 1) * CLEN)
                            nc.vector.tensor_tensor_scan(
                                call[:, cs], gate[:, 0:CLEN], gate[:, 0:CLEN],
                                1.0, OP.mult, OP.bypass)
                            nc.vector.tensor_copy(haux[:, cs], h1[:, 0:CLEN])
                            nc.scalar.copy(gaux[:, cs], ps["g"][:, 0:CLEN])
                        h_prev = h1

            # carry exchange: issue collective ASAP; defer recv consumption
            nc.sync.dma_start(hl_i[:], carry[:])
            nc.gpsimd.collective_compute(
                "AllReduce", OP.add,
                replica_groups=[[0, 1], [2, 3], [4, 5], [6, 7]],
                ins=[hl_i.opt()], outs=[hl_o.opt()])

            # ------------- phase B: rmsnorm scale + output projection -------
            with tc.tile_pool(name="rp", bufs=1) as rp, \
                 tc.tile_pool(name="woq", bufs=1) as woq, \
                 tc.tile_pool(name="sp", bufs=2, space="PSUM") as sp, \
                 tc.tile_pool(name="yp", bufs=6, space="PSUM") as yp, \
                 tc.tile_pool(name="yo", bufs=2) as yo, \
                 tc.tile_pool(name="fx", bufs=2) as fx:
                wo_all = woq.tile([128, DT * ET * 128], BF16, tag="wo")
                for dt in range(DT):
                    nc.sync.dma_start(
                        wo_all[:, dt * D:(dt + 1) * D],
                        wo_d.ap()[dt * 128:(dt + 1) * 128, :])

                # rms chain chunked per 512 so no single slow reciprocal
                # blocks the vector queue; yp bufs=6 lets the tensor engine
                # run ~20us ahead while this pipeline fills
                mrec = rp.tile([128, T], F32, tag="mrec")
                rms = rp.tile([128, T], F32, tag="rms")
                for n in range(T // 512):
                    ns = slice(n * 512, (n + 1) * 512)
                    Sn = sp.tile([128, 512], F32, tag="S")
                    nc.tensor.matmul(Sn[:], ones[:], acc[:, ns],
                                     start=True, stop=True)
                    nc.vector.tensor_scalar(mrec[:, ns], Sn[:], 1.0 / D, EPS,
                                            OP.mult, OP.add)
                    nc.vector.reciprocal(mrec[:, ns], mrec[:, ns])
                    nc.scalar.activation(rms[:, ns], mrec[:, ns], AF.Sqrt)

                def outproj(dt, tbs):
                    for tb2 in tbs:
                        ypt = yp.tile([128, TB], F32, tag="ypt")
                        for et in range(ET):
                            nc.tensor.matmul(
                                ypt[:],
                                wo_all[:, dt * D + et * 128:
                                       dt * D + (et + 1) * 128],
                                ublk[tb2][:, et * TB:(et + 1) * TB],
                                start=(et == 0), stop=(et == ET - 1))
                        ysb = yo.tile([128, TB], F32, tag="ysb")
                        nc.vector.tensor_mul(ysb[:], ypt[:],
                                             rms[:, tb2 * TB:(tb2 + 1) * TB])
                        nc.sync.dma_start(
                            yt_d.ap()[dt * 128:(dt + 1) * 128,
                                      tb2 * TB:(tb2 + 1) * TB], ysb[:])

                # blocks 1..3 first: they don't need the carry fixup
                for dt in range(DT):
                    outproj(dt, (1, 2, 3))

                # consume the AllReduce (emitted late so in-order queues
                # don't stall phase B behind the collective)
                nc.sync.dma_start(recv[:], hl_o[:])
                nc.vector.tensor_sub(recv[:], recv[:], carry[:])
                nc.vector.tensor_scalar(cin[:], recv[:], maskt[:, 0:1], None,
                                        OP.mult)
                for et in range(ET):
                    cs = slice(et * CLEN, (et + 1) * CLEN)
                    hf = fx.tile([128, CLEN], F32, tag="hf")
                    nc.vector.scalar_tensor_tensor(
                        hf[:], call[:, cs], cin[:, et:et + 1], haux[:, cs],
                        OP.mult, OP.add)
                    sf = fx.tile([128, CLEN], F32, tag="sf")
                    nc.scalar.activation(sf[:], hf[:], AF.Sigmoid)
                    hfs = fx.tile([128, CLEN], F32, tag="hfs")
                    nc.vector.tensor_mul(hfs[:], hf[:], sf[:])
                    nc.vector.scalar_tensor_tensor(
                        ublk[0][:, et * TB:et * TB + CLEN],
                        gaux[:, cs], gnw[:, et:et + 1], hfs[:],
                        OP.mult, OP.mult)

                # block 0 last: reads the fixed-up ublk[0]
                for dt in range(DT):
                    outproj(dt, (0,))
    nc.compile()
    return nc


def _get_nc():
    if "nc" not in _CACHE:
        _CACHE["nc"] = _build()
    return _CACHE["nc"]


def _prep(wT):
    """[D, D] fp32 (already W.T) -> contiguous bf16 tiles.

    out[a*128+p, b*128+c] = wT[b*128+p, a*128+c]: block-transposed so a DMA of
    row block `a` yields the [128, DT*128] stationary tile for output block a.
    """
    return np.ascontiguousarray(
        np.asarray(wT, np.float32).reshape(DT, 128, ET, 128)
        .transpose(2, 1, 0, 3).reshape(D, D).astype(ml_dtypes.bfloat16))


def kernel(hidden_states, Wi, Wf, Wg, g_norm_weight, Wo, **_unused):
    nc = _get_nc()
    wi = _prep(np.asarray(Wi, np.float32).T)
    wf = _prep(np.asarray(Wf, np.float32).T)
    wg = _prep(np.asarray(Wg, np.float32).T)
    wo = _prep(np.asarray(Wo, np.float32).T)
    gnw = np.ascontiguousarray(
        np.asarray(g_norm_weight, np.float32).reshape(ET, 128).T)
    in_maps = []
    for c in range(NCORE):
        b, half = c // 2, c % 2
        xt = np.ascontiguousarray(
            np.asarray(hidden_states, np.float32)[b, half * T:(half + 1) * T, :].T
        ).astype(ml_dtypes.bfloat16)
        mask = np.full((128, 1), float(half), np.float32)
        in_maps.append({"xt": xt, "wi": wi, "wf": wf, "wg": wg,
                        "wo": wo, "gnw": gnw, "mask": mask})
    _CACHE["in_maps"] = in_maps
    res = run_bass_kernel_spmd(nc, in_maps, list(range(NCORE))).results
    y = np.empty((B, L, D), np.float32)
    for c in range(NCORE):
        b, half = c // 2, c % 2
        y[b, half * T:(half + 1) * T, :] = res[c]["yt"].T
    return y
